# revision 20
# baseline (speedup 1.0000x reference)
"""Trainium2 Bass kernel for nn_AggregationRebuild_HN (sparse_attention).

Computes, for each of B=512 samples:
    out[b] = sum_j softmax(sim[b] / 0.02)[j] * block_j(b)          # [64, 128]
where block_j(b) are 3 "positive" rows (512 + 3b + j of p_enc_out) and 16
gathered "negative" rows (p_enc_out[negative_index[b, j]]).

Strategy ("scatter-softmax-matmul"):
  * Shard the P*D = 8192 feature axis across 8 cores (1024 features each).
    Every core reads its slice of p_enc_out exactly once (~8.5 MiB) -- the
    roofline-minimal HBM traffic -- instead of re-reading gathered rows.
  * The whole gather + weighted sum becomes a single PE-matmul chain per
    output tile: out[b, :] = sum_k WT[k, b] * pool[k, :], with the K axis
    made of 4 chunks of original rows (negatives), >=1 "replica" chunk
    (extra copies of pool rows for duplicate indices within a sample), and
    3 per-tile chunks of positive rows (block-band structure).
  * WT (the softmax *numerators* scattered into K-slot positions) is
    produced on device by one ACT Exp over a host-built scatter of the
    max-shifted logits (empty slots hold -3e4 -> exp -> 0).  The softmax
    denominator is computed on device from the [512, 19] logits; 1/Z lands
    as a per-partition scale on the PSUM->SBUF copy.
  * Host-side work is index bookkeeping + the standard stable-softmax max
    shift only; exp/sum/normalize and all matvec math run on device.

Hardware constraint that shapes the code: most TRN2 instructions accept a
single sync-wait command, so the program is arranged so every instruction
needs at most one cross-engine wait (dummy "wait absorber" ops pre-observe
sems; DMAs are consolidated so semaphore lanes are not reused and the tail
drain's wait list stays small).
"""

from contextlib import ExitStack

import numpy as np

_B = 512            # bs * n_vars
_P = 64             # patch_num
_D = 128            # d_model
_KP = 3             # k_positive
_KN = 16            # k_negative
_NCORES = 8
_PPC = _P // _NCORES        # patches per core = 8
_PDC = _PPC * _D            # features per core = 1024
_SENT = -3.0e4              # empty-slot sentinel; exp(50 * -3e4) == 0
_SCALE = 50.0               # 1 / temperature
_NTILES = _B // 128         # 4 M-tiles of 128 samples


def _build_host(sim, neg_idx):
    """Index bookkeeping + stable-softmax shift.

    Returns (ssc_ext, rep_rows, n_rep_chunks):
      ssc_ext [128, ncols + 76] f32:
        cols [512c, 512c+512) c<4   : original-row slots (chunk c)
        cols [2048 + 512q, ...)     : replica chunks q
        cols [pos_col0 + 128*(3t+pc), +128): positive band block (t, pc)
        cols [ncols, ncols+76)      : max-shifted logits, [p, t, k] layout
      rep_rows [128 * n_rep_chunks] : pool row content of each replica slot
    """
    sim = np.asarray(sim, np.float32)
    neg_idx = np.asarray(neg_idx).astype(np.int64)
    m = sim.max(axis=1, keepdims=True)
    simsh = np.ascontiguousarray(sim - m, dtype=np.float32)  # [B, 19]

    # Duplicate handling: the d-th extra occurrence of pool row r within one
    # sample needs its own K slot whose rhs content is (a copy of) row r.
    occ = {}
    dup_rank = np.zeros((_B, _KN), np.int64)
    for b in range(_B):
        seen = {}
        for j in range(_KN):
            r = int(neg_idx[b, j])
            dup_rank[b, j] = seen.get(r, 0)
            seen[r] = seen.get(r, 0) + 1
        for r, c in seen.items():
            if c - 1 > occ.get(r, 0):
                occ[r] = c - 1
    rep_slot = {}
    rep_rows = []
    for r in sorted(occ):
        for d in range(occ[r]):
            rep_slot[(r, d)] = len(rep_rows)
            rep_rows.append(r)
    n_rep_chunks = max(1, -(-len(rep_rows) // 128))
    rep_rows = rep_rows + [0] * (n_rep_chunks * 128 - len(rep_rows))

    pos_col0 = 2048 + 512 * n_rep_chunks
    ncols = pos_col0 + 128 * 3 * _NTILES
    ssc_ext = np.full((128, ncols + _NTILES * 19), _SENT, np.float32)

    for b in range(_B):
        t, bl = b >> 7, b & 127
        for j in range(_KP):
            slot = 3 * bl + j
            pc, kl = slot >> 7, slot & 127
            ssc_ext[kl, pos_col0 + 128 * (3 * t + pc) + bl] = simsh[b, j]
        for j in range(_KN):
            r = int(neg_idx[b, j])
            d = int(dup_rank[b, j])
            if d == 0:
                ssc_ext[r & 127, 512 * (r >> 7) + b] = simsh[b, _KP + j]
            else:
                q = rep_slot[(r, d - 1)]
                ssc_ext[q & 127, 2048 + 512 * (q >> 7) + b] = simsh[b, _KP + j]
    # shifted logits for the softmax denominator: [p, t, k] layout
    ssc_ext[:, ncols:] = simsh.reshape(_NTILES, 128, 19).transpose(1, 0, 2).reshape(128, -1)
    return ssc_ext, np.array(rep_rows, np.int64), n_rep_chunks


def _kernel_body(ctx, tc, out_ap, poolhi_ap, poollo_ap, ssc_ap, n_rep_chunks):
    import concourse.mybir as mybir

    nc = tc.nc
    f32 = mybir.dt.float32
    bf16 = mybir.dt.bfloat16
    AF = mybir.ActivationFunctionType
    n_chunks = 16 + n_rep_chunks
    pos_col0 = 2048 + 512 * n_rep_chunks
    ncols = pos_col0 + 128 * 3 * _NTILES

    const = ctx.enter_context(tc.tile_pool(name="const", bufs=1))
    psum_pool = ctx.enter_context(tc.tile_pool(name="psum", bufs=4, space="PSUM"))

    # --- scattered logits (+ appended plain logits), one DMA ------------
    ssc = const.tile([128, ncols + _NTILES * 19], f32, tag="ssc")
    nc.sync.dma_start(out=ssc[:], in_=ssc_ap[:, :])

    # softmax numerators, scattered into K-slot positions; then a lossless
    # bf16 hi/lo split (wt = whi + wlo) for the 3-term bf16 matmul scheme:
    #   W @ P = Whi @ Phi + Whi @ Plo + Wlo @ Phi   (+ O(2^-18) dropped term)
    wt = const.tile([128, ncols], f32, tag="wt")
    nc.scalar.activation(out=wt[:], in_=ssc[:, :ncols], func=AF.Exp, scale=_SCALE)
    whi = const.tile([128, ncols], bf16, tag="whi")
    nc.scalar.activation(out=whi[:], in_=wt[:], func=AF.Copy)
    wlo = const.tile([128, ncols], bf16, tag="wlo")
    nc.vector.tensor_sub(wlo[:], wt[:], whi[:])

    # softmax denominator Z[b] and 1/Z
    rz = const.tile([128, _NTILES], f32, tag="rz")
    for t in range(_NTILES):
        e = const.tile([128, 19], f32, tag=f"e{t}", name=f"e{t}")
        nc.scalar.activation(
            out=e[:],
            in_=ssc[:, ncols + 19 * t : ncols + 19 * (t + 1)],
            func=AF.Exp,
            scale=_SCALE,
        )
        z = const.tile([128, 1], f32, tag=f"z{t}", name=f"z{t}")
        nc.vector.reduce_sum(out=z[:], in_=e[:], axis=mybir.AxisListType.X)
        nc.vector.reciprocal(out=rz[:, t : t + 1], in_=z[:])

    # --- pool (host-split bf16 hi/lo), 3 DMAs per half -------------------
    # DRAM row order (host-arranged): negatives (chunks 0-3), replicas
    # (chunks 4..4+R-1), positives (chunks 4+R..15+R).
    nr = n_rep_chunks
    bounds = [0, 4 + nr, 4 + nr + 6, n_chunks]
    pool_sb = {}
    for nm, ap in (("hi", poolhi_ap), ("lo", poollo_ap)):
        pool_sb[nm] = const.tile(
            [128, n_chunks * _PDC], bf16, tag=f"pool_{nm}", name=f"pool_{nm}"
        )
        view = ap.rearrange("(c p) n -> c p n", p=128)
        for k0, k1 in zip(bounds[:-1], bounds[1:]):
            nc.sync.dma_start(
                out=pool_sb[nm][:, _PDC * k0 : _PDC * k1].rearrange(
                    "p (c n) -> p c n", n=_PDC
                ),
                in_=view[k0:k1].rearrange("c p n -> p c n"),
            )

    def chunk(nm, k):
        return pool_sb[nm][:, _PDC * k : _PDC * (k + 1)]

    # --- matmul chains (24 bf16 MMs per (t,h), N=512) + 1/Z scale --------
    out_sb = const.tile([128, _NTILES * _PDC], f32, tag="out_sb")
    out_view = out_ap.rearrange("(t p) n -> t p n", p=128)
    for t in range(_NTILES):
        cols = [512 * c + 128 * t for c in range(4)]
        cols += [2048 + 512 * q + 128 * t for q in range(n_rep_chunks)]
        cols += [pos_col0 + 128 * (3 * t + pc) for pc in range(3)]
        pks = list(range(4)) + [4 + q for q in range(n_rep_chunks)] + [
            4 + n_rep_chunks + 3 * t + pc for pc in range(3)
        ]
        terms = [(whi, "hi"), (whi, "lo"), (wlo, "hi")]
        for h in range(2):
            ps = psum_pool.tile(
                [128, 512], f32, tag="ps", name=f"ps{t}{h}", bufs=8
            )
            n_mm = len(terms) * len(cols)
            i = 0
            for w_tile, p_nm in terms:
                for wc, pk in zip(cols, pks):
                    nc.tensor.matmul(
                        ps[:],
                        lhsT=w_tile[:, wc : wc + 128],
                        rhs=chunk(p_nm, pk)[:, 512 * h : 512 * (h + 1)],
                        start=(i == 0),
                        stop=(i == n_mm - 1),
                    )
                    i += 1
            nc.scalar.activation(
                out=out_sb[:, _PDC * t + 512 * h : _PDC * t + 512 * (h + 1)],
                in_=ps[:],
                func=AF.Copy,
                scale=rz[:, t : t + 1],
            )
        nc.sync.dma_start(
            out=out_view[t, :, :], in_=out_sb[:, _PDC * t : _PDC * (t + 1)]
        )


_prog_cache = {}


def _get_program(n_rep_chunks):
    if n_rep_chunks in _prog_cache:
        return _prog_cache[n_rep_chunks]
    import concourse.bacc as bacc
    import concourse.mybir as mybir
    import concourse.tile as tile

    nc = bacc.Bacc(
        "TRN2",
        target_bir_lowering=False,
        debug=False,
        enable_asserts=False,
        num_devices=_NCORES,
    )
    n_pool_rows = 2048 + 128 * n_rep_chunks
    pos_col0 = 2048 + 512 * n_rep_chunks
    ncols = pos_col0 + 128 * 3 * _NTILES
    f32 = mybir.dt.float32
    bf16 = mybir.dt.bfloat16
    poolhi_ap = nc.dram_tensor(
        "poolhi", [n_pool_rows, _PDC], bf16, kind="ExternalInput"
    ).ap()
    poollo_ap = nc.dram_tensor(
        "poollo", [n_pool_rows, _PDC], bf16, kind="ExternalInput"
    ).ap()
    ssc_ap = nc.dram_tensor(
        "ssc", [128, ncols + _NTILES * 19], f32, kind="ExternalInput"
    ).ap()
    out_ap = nc.dram_tensor("out", [_B, _PDC], f32, kind="ExternalOutput").ap()
    with tile.TileContext(nc) as tc:
        with ExitStack() as ctx:
            _kernel_body(ctx, tc, out_ap, poolhi_ap, poollo_ap, ssc_ap, n_rep_chunks)
    nc.compile()
    _prog_cache[n_rep_chunks] = nc
    return nc


def _prepare(similarity_matrix, p_enc_out, negative_index):
    sim = np.asarray(similarity_matrix, np.float32)
    pool = np.asarray(p_enc_out, np.float32)
    assert sim.shape == (_B, _KP + _KN), sim.shape
    assert pool.shape == (_B * (1 + _KP), _P, _D), pool.shape
    ssc_ext, rep_rows, n_rep_chunks = _build_host(sim, negative_index)
    in_maps = []
    import ml_dtypes

    for c in range(_NCORES):
        sl = pool[:, _PPC * c : _PPC * (c + 1), :].reshape(-1, _PDC)
        rep = pool[rep_rows, _PPC * c : _PPC * (c + 1), :].reshape(-1, _PDC)
        # row order: negatives, replicas, positives
        pc = np.concatenate([sl[:_B], rep, sl[_B:]], axis=0)
        # lossless-enough bf16 split: pc ~= hi + lo (hi = round(pc),
        # lo = round(pc - hi); residual is O(2^-16) relative)
        hi = pc.astype(ml_dtypes.bfloat16)
        lo = (pc - hi.astype(np.float32)).astype(ml_dtypes.bfloat16)
        in_maps.append(
            {
                "poolhi": np.ascontiguousarray(hi),
                "poollo": np.ascontiguousarray(lo),
                "ssc": ssc_ext,
            }
        )
    return in_maps, n_rep_chunks


def _postprocess(results):
    outs = [r["out"].reshape(_B, _PPC, _D) for r in results]
    return np.ascontiguousarray(np.concatenate(outs, axis=1))


def kernel(similarity_matrix, p_enc_out, negative_index, **_unused):
    from concourse.bass_utils import run_bass_kernel_spmd

    in_maps, n_rep_chunks = _prepare(similarity_matrix, p_enc_out, negative_index)
    nc = _get_program(n_rep_chunks)
    res = run_bass_kernel_spmd(nc, in_maps, core_ids=list(range(_NCORES)))
    return _postprocess(res.results)


if __name__ == "__main__":
    # smoke test with random data (no reference available here)
    rng = np.random.default_rng(0)
    sim = rng.standard_normal((_B, _KP + _KN), dtype=np.float32)
    pool = rng.standard_normal((_B * (1 + _KP), _P, _D), dtype=np.float32)
    idx = rng.integers(0, _B, size=(_B, _KN))
    out = kernel(similarity_matrix=sim, p_enc_out=pool, negative_index=idx)
    print("out", out.shape, out.dtype, float(np.abs(out).mean()))


# revision 21
# speedup vs baseline: 1.0299x; 1.0299x over previous
"""Trainium2 Bass kernel for nn_AggregationRebuild_HN (sparse_attention).

Computes, for each of B=512 samples:
    out[b] = sum_j softmax(sim[b] / 0.02)[j] * block_j(b)          # [64, 128]
where block_j(b) are 3 "positive" rows (512 + 3b + j of p_enc_out) and 16
gathered "negative" rows (p_enc_out[negative_index[b, j]]).

Strategy ("scatter-softmax-matmul"):
  * Shard the P*D = 8192 feature axis across 8 cores (1024 features each).
    Every core reads its slice of p_enc_out exactly once (~8.5 MiB) -- the
    roofline-minimal HBM traffic -- instead of re-reading gathered rows.
  * The whole gather + weighted sum becomes PE-matmul chains:
    out[b, :] = sum_k WT[k, b] * pool[k, :].  The K axis per 128-sample
    tile: 4 chunks of original rows (negatives), >=1 "replica" chunk
    (extra copies of pool rows for duplicate indices within one sample),
    plus 4 column-group-packed positive chunks (K=96, M=32 each) that run
    CONCURRENTLY on the PE via tile_position col-tiling.
  * WT (softmax *numerators* scattered into K-slot positions) is produced
    on device by one ACT Exp over a host-built scatter of the max-shifted
    logits (empty slots hold -3e4 -> exp -> 0).  The softmax denominator
    is computed on device from the [512, 19] logits; 1/Z lands as a
    per-partition scale on the PSUM->SBUF copy.
  * Matmuls run as a 3-term bf16 hi/lo decomposition (full ~1e-6 fp32
    accuracy at 2x the fp32 PE rate):
        W @ P = Whi @ Phi + Whi @ Plo + Wlo @ Phi  (+ O(2^-18) dropped)
    The pool is split hi/lo losslessly on the host (same total bytes as
    fp32); W is split on device after the exp.
  * Host-side work is index bookkeeping, dtype splitting, and the standard
    stable-softmax max shift only; exp/sum/normalize and all matvec math
    run on device.
"""

from contextlib import ExitStack

import numpy as np

_B = 512            # bs * n_vars
_P = 64             # patch_num
_D = 128            # d_model
_KP = 3             # k_positive
_KN = 16            # k_negative
_NCORES = 8
_PPC = _P // _NCORES        # patches per core = 8
_PDC = _PPC * _D            # features per core = 1024
_SENT = -3.0e4              # empty-slot sentinel; exp(50 * -3e4) == 0
_SCALE = 50.0               # 1 / temperature
_NTILES = _B // 128         # 4 M-tiles of 128 samples
_NPOSBLK = 4 * _NTILES      # 16 positive blocks (K=96, M=32 each)


def _build_host(sim, neg_idx):
    """Index bookkeeping + stable-softmax shift.

    ssc layout [128, ncols_slots + 76] f32 (sentinel -3e4 in empty cells):
      cols [512c, 512c+512), c<4      : original-row slots (neg chunk c)
      cols [2048 + 512q, ...)         : replica chunks q
      cols [pos0 + 32*blk, +32)       : positive block blk=(4t+pc),
                                        row 3*(b%32)+j, col b%32  (K=96,M=32)
      cols [ncols_slots, +76)         : max-shifted logits, [p, t, k] layout
    """
    sim = np.asarray(sim, np.float32)
    neg_idx = np.asarray(neg_idx).astype(np.int64)
    m = sim.max(axis=1, keepdims=True)
    simsh = np.ascontiguousarray(sim - m, dtype=np.float32)  # [B, 19]

    # Duplicate handling: the d-th extra occurrence of pool row r within one
    # sample needs its own K slot whose rhs content is (a copy of) row r.
    occ = {}
    dup_rank = np.zeros((_B, _KN), np.int64)
    for b in range(_B):
        seen = {}
        for j in range(_KN):
            r = int(neg_idx[b, j])
            dup_rank[b, j] = seen.get(r, 0)
            seen[r] = seen.get(r, 0) + 1
        for r, c in seen.items():
            if c - 1 > occ.get(r, 0):
                occ[r] = c - 1
    rep_slot = {}
    rep_rows = []
    for r in sorted(occ):
        for d in range(occ[r]):
            rep_slot[(r, d)] = len(rep_rows)
            rep_rows.append(r)
    n_rep_chunks = max(1, -(-len(rep_rows) // 128))
    rep_rows = rep_rows + [0] * (n_rep_chunks * 128 - len(rep_rows))

    pos0 = 2048 + 512 * n_rep_chunks
    ncols_slots = pos0 + 32 * _NPOSBLK
    ssc = np.full((128, ncols_slots + _NTILES * 19), _SENT, np.float32)

    for b in range(_B):
        t, bl = b >> 7, b & 127
        pc, mm = bl >> 5, bl & 31
        blk = 4 * t + pc
        for j in range(_KP):
            ssc[3 * mm + j, pos0 + 32 * blk + mm] = simsh[b, j]
        for j in range(_KN):
            r = int(neg_idx[b, j])
            d = int(dup_rank[b, j])
            if d == 0:
                ssc[r & 127, 512 * (r >> 7) + b] = simsh[b, _KP + j]
            else:
                q = rep_slot[(r, d - 1)]
                ssc[q & 127, 2048 + 512 * (q >> 7) + b] = simsh[b, _KP + j]
    # shifted logits for the softmax denominator: [p, t, k] layout
    ssc[:, ncols_slots:] = (
        simsh.reshape(_NTILES, 128, 19).transpose(1, 0, 2).reshape(128, -1)
    )
    return ssc, np.array(rep_rows, np.int64), n_rep_chunks


def _kernel_body(ctx, tc, out_ap, poolhi_ap, poollo_ap, ssc_ap, n_rep_chunks):
    import concourse.mybir as mybir

    nc = tc.nc
    f32 = mybir.dt.float32
    bf16 = mybir.dt.bfloat16
    AF = mybir.ActivationFunctionType
    nr = n_rep_chunks
    n_sq = 4 + nr                      # serial (neg + replica) chunks
    nrr = 512 + 128 * nr               # rows before the positive region
    pos0 = 2048 + 512 * nr
    ncols_slots = pos0 + 32 * _NPOSBLK

    const = ctx.enter_context(tc.tile_pool(name="const", bufs=1))
    psum_pool = ctx.enter_context(tc.tile_pool(name="psum", bufs=8, space="PSUM"))

    # --- scattered logits (+ appended plain logits), one DMA ------------
    ssc = const.tile([128, ncols_slots + _NTILES * 19], f32, tag="ssc")
    nc.sync.dma_start(out=ssc[:], in_=ssc_ap[:, :])

    # softmax numerators, then the bf16 hi/lo split (wt = whi + wlo)
    wt = const.tile([128, ncols_slots], f32, tag="wt")
    nc.scalar.activation(out=wt[:], in_=ssc[:, :ncols_slots], func=AF.Exp, scale=_SCALE)
    whi = const.tile([128, ncols_slots], bf16, tag="whi")
    nc.scalar.activation(out=whi[:], in_=wt[:], func=AF.Copy)
    wlo = const.tile([128, ncols_slots], bf16, tag="wlo")
    nc.vector.tensor_sub(wlo[:], wt[:], whi[:])

    # softmax denominator Z[b] and 1/Z
    rz = const.tile([128, _NTILES], f32, tag="rz")
    for t in range(_NTILES):
        e = const.tile([128, 19], f32, tag=f"e{t}", name=f"e{t}")
        nc.scalar.activation(
            out=e[:],
            in_=ssc[:, ncols_slots + 19 * t : ncols_slots + 19 * (t + 1)],
            func=AF.Exp,
            scale=_SCALE,
        )
        z = const.tile([128, 1], f32, tag=f"z{t}", name=f"z{t}")
        nc.vector.reduce_sum(out=z[:], in_=e[:], axis=mybir.AxisListType.X)
        nc.vector.reciprocal(out=rz[:, t : t + 1], in_=z[:])

    # --- pool (host-split bf16 hi/lo) ------------------------------------
    # DRAM row order: negatives (4x128), replicas (nr x 128), positives
    # (16 x 96, natural p_enc_out order).  negrep tiles are [128, *]; the
    # positive region is viewed/stored with 96 partitions.
    negrep_sb, pos_sb = {}, {}
    for nm, ap in (("hi", poolhi_ap), ("lo", poollo_ap)):
        negrep_sb[nm] = const.tile(
            [128, n_sq * _PDC], bf16, tag=f"negrep_{nm}", name=f"negrep_{nm}"
        )
        pos_sb[nm] = const.tile(
            [96, _NPOSBLK * _PDC], bf16, tag=f"pos_{nm}", name=f"pos_{nm}"
        )

    def load_negrep(nm, ap):
        view = ap[: nrr * _PDC].rearrange("(c p n) -> c p n", p=128, n=_PDC)
        nc.sync.dma_start(
            out=negrep_sb[nm][:].rearrange("p (c n) -> p c n", n=_PDC),
            in_=view.rearrange("c p n -> p c n"),
        )

    def load_pos(nm, ap, b0, b1):
        view = ap[nrr * _PDC :].rearrange("(c p n) -> c p n", p=96, n=_PDC)
        nc.sync.dma_start(
            out=pos_sb[nm][:, _PDC * b0 : _PDC * b1].rearrange(
                "p (c n) -> p c n", n=_PDC
            ),
            in_=view[b0:b1].rearrange("c p n -> p c n"),
        )

    # issue order tuned so the first matmul chains are fed just in time
    load_negrep("hi", poolhi_ap)
    load_pos("hi", poolhi_ap, 0, 8)
    load_negrep("lo", poollo_ap)
    load_pos("lo", poollo_ap, 0, 8)
    load_pos("hi", poolhi_ap, 8, _NPOSBLK)
    load_pos("lo", poollo_ap, 8, _NPOSBLK)

    # --- matmul chains + 1/Z scale ---------------------------------------
    # per (t, h): 3 terms x (n_sq serial MMs + 4 concurrent col-tiled pos MMs)
    out_sb = const.tile([128, _NTILES * _PDC], f32, tag="out_sb")
    out_view = out_ap.rearrange("(t p) n -> t p n", p=128)
    for t in range(_NTILES):
        for h in range(2):
            ps = psum_pool.tile([128, 512], f32, tag="ps", name=f"ps{t}{h}", bufs=8)
            terms = [(whi, "hi"), (whi, "lo"), (wlo, "hi")]
            n_mm = len(terms) * (n_sq + 4)
            i = 0
            for w_tile, p_nm in terms:
                for c in range(4):
                    nc.tensor.matmul(
                        ps[:],
                        lhsT=w_tile[:, 512 * c + 128 * t : 512 * c + 128 * (t + 1)],
                        rhs=negrep_sb[p_nm][:, _PDC * c + 512 * h : _PDC * c + 512 * (h + 1)],
                        start=(i == 0),
                        stop=(i == n_mm - 1),
                    )
                    i += 1
                for q in range(nr):
                    nc.tensor.matmul(
                        ps[:],
                        lhsT=w_tile[:, 2048 + 512 * q + 128 * t : 2048 + 512 * q + 128 * (t + 1)],
                        rhs=negrep_sb[p_nm][
                            :, _PDC * (4 + q) + 512 * h : _PDC * (4 + q) + 512 * (h + 1)
                        ],
                        start=(i == 0),
                        stop=(i == n_mm - 1),
                    )
                    i += 1
                for pc in range(4):
                    blk = 4 * t + pc
                    nc.tensor.matmul(
                        ps[32 * pc : 32 * (pc + 1), :],
                        lhsT=w_tile[:96, pos0 + 32 * blk : pos0 + 32 * (blk + 1)],
                        rhs=pos_sb[p_nm][
                            :, _PDC * blk + 512 * h : _PDC * blk + 512 * (h + 1)
                        ],
                        start=(i == 0),
                        stop=(i == n_mm - 1),
                        tile_position=(0, 32 * pc),
                    )
                    i += 1
            nc.scalar.activation(
                out=out_sb[:, _PDC * t + 512 * h : _PDC * t + 512 * (h + 1)],
                in_=ps[:],
                func=AF.Copy,
                scale=rz[:, t : t + 1],
            )
        # store on the ACT HWDGE ring so it doesn't queue behind input DMAs
        nc.scalar.dma_start(
            out=out_view[t, :, :], in_=out_sb[:, _PDC * t : _PDC * (t + 1)]
        )


_prog_cache = {}


def _get_program(n_rep_chunks):
    if n_rep_chunks in _prog_cache:
        return _prog_cache[n_rep_chunks]
    import concourse.bacc as bacc
    import concourse.mybir as mybir
    import concourse.tile as tile

    nc = bacc.Bacc(
        "TRN2",
        target_bir_lowering=False,
        debug=False,
        enable_asserts=False,
        num_devices=_NCORES,
    )
    n_pool_rows = 2048 + 128 * n_rep_chunks
    pos0 = 2048 + 512 * n_rep_chunks
    ncols = pos0 + 32 * _NPOSBLK + _NTILES * 19
    f32 = mybir.dt.float32
    bf16 = mybir.dt.bfloat16
    # flat [rows*1024] so the 128-row negrep and 96-row positive regions can
    # each be viewed with their own partition factor
    poolhi_ap = nc.dram_tensor(
        "poolhi", [n_pool_rows * _PDC], bf16, kind="ExternalInput"
    ).ap()
    poollo_ap = nc.dram_tensor(
        "poollo", [n_pool_rows * _PDC], bf16, kind="ExternalInput"
    ).ap()
    ssc_ap = nc.dram_tensor("ssc", [128, ncols], f32, kind="ExternalInput").ap()
    out_ap = nc.dram_tensor("out", [_B, _PDC], f32, kind="ExternalOutput").ap()
    with tile.TileContext(nc) as tc:
        with ExitStack() as ctx:
            _kernel_body(ctx, tc, out_ap, poolhi_ap, poollo_ap, ssc_ap, n_rep_chunks)
    nc.compile()
    _prog_cache[n_rep_chunks] = nc
    return nc


def _prepare(similarity_matrix, p_enc_out, negative_index):
    import ml_dtypes

    sim = np.asarray(similarity_matrix, np.float32)
    pool = np.asarray(p_enc_out, np.float32)
    assert sim.shape == (_B, _KP + _KN), sim.shape
    assert pool.shape == (_B * (1 + _KP), _P, _D), pool.shape
    ssc, rep_rows, n_rep_chunks = _build_host(sim, negative_index)
    in_maps = []
    for c in range(_NCORES):
        sl = pool[:, _PPC * c : _PPC * (c + 1), :].reshape(-1, _PDC)
        rep = pool[rep_rows, _PPC * c : _PPC * (c + 1), :].reshape(-1, _PDC)
        # row order: negatives, replicas, positives (natural order)
        pc = np.concatenate([sl[:_B], rep, sl[_B:]], axis=0)
        # lossless-enough bf16 split: pc ~= hi + lo
        hi = pc.astype(ml_dtypes.bfloat16)
        lo = (pc - hi.astype(np.float32)).astype(ml_dtypes.bfloat16)
        in_maps.append(
            {
                "poolhi": np.ascontiguousarray(hi.reshape(-1)),
                "poollo": np.ascontiguousarray(lo.reshape(-1)),
                "ssc": ssc,
            }
        )
    return in_maps, n_rep_chunks


def _postprocess(results):
    outs = [r["out"].reshape(_B, _PPC, _D) for r in results]
    return np.ascontiguousarray(np.concatenate(outs, axis=1))


def kernel(similarity_matrix, p_enc_out, negative_index, **_unused):
    from concourse.bass_utils import run_bass_kernel_spmd

    in_maps, n_rep_chunks = _prepare(similarity_matrix, p_enc_out, negative_index)
    nc = _get_program(n_rep_chunks)
    res = run_bass_kernel_spmd(nc, in_maps, core_ids=list(range(_NCORES)))
    return _postprocess(res.results)


if __name__ == "__main__":
    # smoke test with random data (no reference available here)
    rng = np.random.default_rng(0)
    sim = rng.standard_normal((_B, _KP + _KN), dtype=np.float32)
    pool = rng.standard_normal((_B * (1 + _KP), _P, _D), dtype=np.float32)
    idx = rng.integers(0, _B, size=(_B, _KN))
    out = kernel(similarity_matrix=sim, p_enc_out=pool, negative_index=idx)
    print("out", out.shape, out.dtype, float(np.abs(out).mean()))
